# revision 1
# baseline (speedup 1.0000x reference)
"""Trainium2 Bass kernel for BatchAll triplet loss.

Reference computation (B=512, D=1024):
    pw = img @ sent.T                                  [B, B]
    t[a,p,n] = pw[a,p] - pw[a,n] + margin
    valid[a,p,n] = (lab[a]==lab[p]) & (lab[a]!=lab[n])
    loss = sum(relu(valid*t)) / (count(valid*t > EPS) + EPS)

Strategy: the batch is class-sorted on the host (a pure permutation of the
(image, sentence, label) triples; the loss is permutation invariant), then
anchors are sharded across 8 cores (64 each, C = core*64). After sorting,
the positives of anchor g all live in a contiguous class run within
(g-16, g+16) as long as every class has <= 16 members (checked on host;
dense fallback otherwise). So for anchor a (local), the p-axis can be
restricted to a 64-wide, 32-aligned window inside the core's 128-wide
sentence window [C-32, C+96).

Per core:
    pw rows over full n [64, 512] and over the window [64, 128] (PE).
    w[a,pwin] = pw+margin if same label else -1e30      [64, 128]
    z[a,n]    = -pw       if label differs else -1e30   [64, 512] bf16
    wT = transpose(w)  -> per-anchor bias columns       [128, 64]
    Main loop packs TWO anchors per tile: partitions = 2 x 64-window,
    free = all 512 n.  PE broadcasts the two z rows via a two-hot
    selector matmul; ACT applies relu(z + w) with the stacked window
    bias and accum_out row-sums; DVE counts r > EPS with accum_out.
Host combines the 8 (sum, count) pairs and divides.
"""

import numpy as np
from contextlib import ExitStack

B = 512
D = 1024
NCORES = 8
A = B // NCORES   # 64 anchors per core
KT = D // 128     # 8 contraction tiles
NT = B // 128     # 4 n-tiles per anchor (dense variant)
W = 128           # per-core sentence window width
MARGIN = 0.2
EPS = 1e-16
BIG = 1e30
MAXC_WIN = 21     # windowed variant valid iff max class size <= this

_CACHE = {}


def _lo_local(a):
    """32-aligned offset of anchor a's 64-wide window inside the core's
    128-wide window (anchor a sits at local window position 32 + a)."""
    return 32 * ((a - 15) // 32) + 32


def _build_win():
    """Class-sorted windowed kernel (primary)."""
    import concourse.bass as bass
    import concourse.mybir as mybir
    import concourse.tile as tile
    from concourse import bacc
    from concourse.masks import make_identity

    f32 = mybir.dt.float32
    bf16 = mybir.dt.float16
    BIGW = 30000.0
    Alu = mybir.AluOpType
    Act = mybir.ActivationFunctionType
    Ax = mybir.AxisListType

    nc = bacc.Bacc("TRN2", target_bir_lowering=False, debug=False,
                   num_devices=NCORES)

    NG = 21  # groups of 3 anchors (anchor 63 = tail)
    packT_d = nc.dram_tensor("packT", [D, A + B], bf16, kind="ExternalInput")
    labf_d = nc.dram_tensor("labf", [B], bf16, kind="ExternalInput")
    labc_d = nc.dram_tensor("labc", [A], f32, kind="ExternalInput")
    out_d = nc.dram_tensor("out", [2], f32, kind="ExternalOutput")
    wdram_d = nc.dram_tensor("wdram", [A, W], bf16)

    with tile.TileContext(nc) as tc:
        with ExitStack() as ctx:
            singles = ctx.enter_context(tc.tile_pool(name="singles", bufs=1))
            rpool = ctx.enter_context(tc.tile_pool(name="rpool", bufs=6))
            mpool = ctx.enter_context(tc.tile_pool(name="mpool", bufs=6))
            spsum = ctx.enter_context(
                tc.tile_pool(name="spsum", bufs=1, space="PSUM"))
            wpsum = ctx.enter_context(
                tc.tile_pool(name="wpsum", bufs=3, space="PSUM"))

            # ---- constants ----
            ones_r = singles.tile([1, A], bf16)
            nc.vector.memset(ones_r, 1.0)
            ones_c = singles.tile([128, 1], f32)
            nc.vector.memset(ones_c, 1.0)
            # selbias[k, ar, h, v] = 1 iff k == ar + 21*h: each matmul tile
            # stacks THREE anchors (ar, ar+21, ar+42), each with a 42-wide
            # window. Row 64 is filled later with the window bias w.
            # Anchor 63 is handled by a separate small tail tile.
            selbias = singles.tile([A + 1, NG, 3, 42], bf16)
            nc.gpsimd.memset(selbias, 0.0)
            nc.gpsimd.affine_select(
                out=selbias, in_=selbias, compare_op=mybir.AluOpType.not_equal,
                fill=1.0, base=0, pattern=[[-1, NG], [-21, 3], [0, 42]],
                channel_multiplier=1)
            selbias_t = singles.tile([A + 1, 42], bf16)
            nc.gpsimd.memset(selbias_t, 0.0)
            nc.gpsimd.affine_select(
                out=selbias_t, in_=selbias_t,
                compare_op=mybir.AluOpType.not_equal, fill=1.0, base=-63,
                pattern=[[0, 42]], channel_multiplier=1)

            # ---- loads: labels first (they gate the DVE mask chain),
            # packT split per k-tile over two queues so PE starts early ----
            lab_row = singles.tile([1, B], bf16)
            nc.scalar.dma_start(
                out=lab_row, in_=labf_d.ap().rearrange("(o b) -> o b", o=1))
            labc_col = singles.tile([A, 1], f32)
            nc.scalar.dma_start(
                out=labc_col, in_=labc_d.ap().rearrange("(a o) -> a o", o=1))
            packT = singles.tile([128, KT, A + B], bf16)
            packT_v = packT_d.ap().rearrange("(t p) m -> p t m", p=128)
            for kt in range(KT):
                eng = nc.sync if kt % 2 == 0 else nc.scalar
                eng.dma_start(out=packT[:, kt, :], in_=packT_v[:, kt, :])
            imgT = packT[:, :, 0:A]
            sentT = packT[:, :, A:A + B]

            # ---- label broadcast + masks (first PE matmul; DVE runs the
            # mask chain while the pairwise matmuls stream in) ----
            labB_ps = spsum.tile([A, B], f32, tag="sB")
            nc.tensor.matmul(labB_ps, lhsT=ones_r, rhs=lab_row)
            eqP = singles.tile([A, B], f32)
            nc.vector.tensor_scalar(eqP, labB_ps, labc_col, None, Alu.is_equal)
            eqW = eqP[:, 0:W]
            penW = singles.tile([A, W], f32)
            nc.vector.tensor_scalar(penW, eqW, 1.0, BIGW, Alu.subtract, Alu.mult)
            negneq = singles.tile([A, B], f32)
            nc.vector.tensor_scalar(negneq, eqP, 1.0, -1.0, Alu.subtract,
                                    Alu.mult)

            # ---- pairwise rows (sentT pre-rotated: window = cols 0..W) ----
            pw_ps = spsum.tile([A, B], f32, tag="sA")
            for kt in range(KT):
                nc.tensor.matmul(pw_ps, lhsT=imgT[:, kt, :], rhs=sentT[:, kt, :],
                                 start=(kt == 0), stop=(kt == KT - 1))

            # ---- w rows first (the selbias DMAs gate the main loop):
            # w = (pw+margin)*eqW + (eqW-1)*BIGW over the window ----
            w_win = singles.tile([A, W], f32)
            nc.vector.scalar_tensor_tensor(w_win, pw_ps[:, 0:W], MARGIN, eqW,
                                           Alu.add, Alu.mult)
            w_bf = singles.tile([A, W], bf16)
            nc.vector.tensor_add(w_bf, w_win, penW)
            # fold w into the selector: selbias row 64 carries the window
            # bias, matched by the ones row 64 of zext, so each matmul
            # emits t = z + w directly into PSUM. Anchor a's 42-wide window
            # starts at local column 11+a (diagonal), so bounce w through
            # DRAM and gather with an affine AP.
            nc.sync.dma_start(out=wdram_d.ap(), in_=w_bf)
            diag = bass.AP(tensor=wdram_d, offset=11,
                           ap=[[0, 1], [129, NG], [21 * 129, 3], [1, 42]])
            nc.sync.dma_start(out=selbias[64:65, :, :, :], in_=diag)
            tail = bass.AP(tensor=wdram_d, offset=63 * 129 + 11,
                           ap=[[0, 1], [1, 42]])
            nc.gpsimd.dma_start(out=selbias_t[64:65, :], in_=tail)

            # ---- z rows: zext[0:A] = -pw*(1-eqP) - BIGW*eqP ----
            z1 = singles.tile([A, B], f32)
            nc.vector.scalar_tensor_tensor(z1, pw_ps, -1.0, negneq,
                                           Alu.mult, Alu.mult)
            zext = singles.tile([A + 1, B], bf16)
            nc.vector.scalar_tensor_tensor(zext[0:A, :], eqP, -BIGW, z1,
                                           Alu.mult, Alu.add)
            nc.vector.memset(zext[A:A + 1, :], 1.0)

            # ---- accumulators ----
            NCOL = 12  # 10 dual-group + 1 single-group + 1 tail column
            Sacc = singles.tile([128, NCOL], f32)
            Cacc = singles.tile([128, NCOL], f32)
            nc.vector.memset(Sacc, 0.0)
            nc.vector.memset(Cacc, 0.0)
            MP = 3 * 42  # 126 partitions per group tile

            # ---- main loop: 6 anchors / 2 PSUM banks per iteration ----
            for i in range(10):
                zb_ps = wpsum.tile([128, 2, B], f32)
                for u in range(2):
                    nc.tensor.matmul(zb_ps[0:MP, u, :],
                                     lhsT=selbias[:, 2 * i + u, :, :],
                                     rhs=zext)
                r = rpool.tile([128, 2, B], bf16)
                nc.scalar.activation(
                    out=r[0:MP], in_=zb_ps[0:MP], func=Act.Relu, bias=0.0,
                    scale=1.0, accum_out=Sacc[0:MP, i:i + 1])
                m = mpool.tile([128, 2, B], bf16)
                nc.vector.tensor_scalar(
                    m[0:MP], zb_ps[0:MP], EPS, None, Alu.is_gt, Alu.add,
                    accum_out=Cacc[0:MP, i:i + 1])
            # group 20 (single) and the anchor-63 tail share one iteration
            zb_ps = wpsum.tile([128, 2, B], f32)
            nc.tensor.matmul(zb_ps[0:MP, 0, :], lhsT=selbias[:, 20, :, :],
                             rhs=zext)
            nc.tensor.matmul(zb_ps[0:42, 1, :], lhsT=selbias_t, rhs=zext)
            r = rpool.tile([128, 2, B], bf16)
            nc.scalar.activation(
                out=r[0:MP, 0], in_=zb_ps[0:MP, 0], func=Act.Relu, bias=0.0,
                scale=1.0, accum_out=Sacc[0:MP, 10:11])
            nc.scalar.activation(
                out=r[0:42, 1], in_=zb_ps[0:42, 1], func=Act.Relu, bias=0.0,
                scale=1.0, accum_out=Sacc[0:42, 11:12])
            m = mpool.tile([128, 2, B], bf16)
            nc.vector.tensor_scalar(
                m[0:MP, 0], zb_ps[0:MP, 0], EPS, None, Alu.is_gt, Alu.add,
                accum_out=Cacc[0:MP, 10:11])
            nc.vector.tensor_scalar(
                m[0:42, 1], zb_ps[0:42, 1], EPS, None, Alu.is_gt, Alu.add,
                accum_out=Cacc[0:42, 11:12])

            # ---- final reduce ----
            SC = singles.tile([128, 2], f32)
            nc.vector.tensor_reduce(SC[:, 0:1], Sacc, Ax.X, Alu.add)
            nc.vector.tensor_reduce(SC[:, 1:2], Cacc, Ax.X, Alu.add)
            fin_ps = spsum.tile([2, 1], f32, tag="sA")
            nc.tensor.matmul(fin_ps, lhsT=SC, rhs=ones_c)
            fin_sb = singles.tile([2, 1], f32)
            nc.scalar.copy(fin_sb, fin_ps)
            nc.sync.dma_start(
                out=out_d.ap().rearrange("(p o) -> p o", o=1), in_=fin_sb)

    nc.compile()
    return nc


def _build_dense():
    """Dense fallback (no class-size assumption)."""
    import concourse.mybir as mybir
    import concourse.tile as tile
    from concourse import bacc
    from concourse.masks import make_identity

    f32 = mybir.dt.float32
    bf16 = mybir.dt.bfloat16
    Alu = mybir.AluOpType
    Act = mybir.ActivationFunctionType
    Ax = mybir.AxisListType

    nc = bacc.Bacc("TRN2", target_bir_lowering=False, debug=False,
                   num_devices=NCORES)

    imgT_d = nc.dram_tensor("imgT", [D, A], f32, kind="ExternalInput")
    sentT_d = nc.dram_tensor("sentT", [D, B], f32, kind="ExternalInput")
    labf_d = nc.dram_tensor("labf", [B], bf16, kind="ExternalInput")
    labc_d = nc.dram_tensor("labc", [A], f32, kind="ExternalInput")
    out_d = nc.dram_tensor("out", [2], f32, kind="ExternalOutput")
    wdram_d = nc.dram_tensor("wdram", [A, W], bf16)

    with tile.TileContext(nc) as tc:
        with ExitStack() as ctx:
            singles = ctx.enter_context(tc.tile_pool(name="singles", bufs=1))
            rpool = ctx.enter_context(tc.tile_pool(name="rpool", bufs=6))
            mpool = ctx.enter_context(tc.tile_pool(name="mpool", bufs=6))
            spsum = ctx.enter_context(
                tc.tile_pool(name="spsum", bufs=1, space="PSUM"))
            wpsum = ctx.enter_context(
                tc.tile_pool(name="wpsum", bufs=3, space="PSUM"))

            ones_r = singles.tile([1, 128], f32)
            nc.vector.memset(ones_r, 1.0)
            ones_c = singles.tile([128, 1], f32)
            nc.vector.memset(ones_c, 1.0)
            ident = singles.tile([64, 64], f32)
            make_identity(nc, ident)

            imgT = singles.tile([128, KT, A], f32)
            nc.sync.dma_start(
                out=imgT, in_=imgT_d.ap().rearrange("(t p) m -> p t m", p=128))
            sentT = singles.tile([128, KT, B], f32)
            nc.sync.dma_start(
                out=sentT, in_=sentT_d.ap().rearrange("(t p) m -> p t m", p=128))
            lab_row = singles.tile([1, B], f32)
            nc.sync.dma_start(
                out=lab_row, in_=labf_d.ap().rearrange("(o b) -> o b", o=1))
            labc_col = singles.tile([A, 1], f32)
            nc.sync.dma_start(
                out=labc_col, in_=labc_d.ap().rearrange("(a o) -> a o", o=1))

            pw_ps = spsum.tile([A, B], f32)
            for kt in range(KT):
                nc.tensor.matmul(pw_ps, lhsT=imgT[:, kt, :], rhs=sentT[:, kt, :],
                                 start=(kt == 0), stop=(kt == KT - 1))

            labB_ps = spsum.tile([A, B], f32)
            nc.tensor.matmul(labB_ps, lhsT=ones_r[:, :A], rhs=lab_row)
            eqP = singles.tile([A, B], f32)
            nc.vector.tensor_scalar(eqP, labB_ps, labc_col, None, Alu.is_equal)
            penP = singles.tile([A, B], f32)
            nc.vector.tensor_scalar(penP, eqP, 1.0, BIG, Alu.subtract, Alu.mult)
            penN = singles.tile([A, B], f32)
            nc.vector.tensor_scalar(penN, eqP, -BIG, None, Alu.mult)

            w = singles.tile([A, B], f32)
            nc.vector.tensor_scalar(w, pw_ps, MARGIN, None, Alu.add)
            nc.vector.tensor_mul(w, w, eqP)
            nc.vector.tensor_add(w, w, penP)
            negneq = singles.tile([A, B], f32)
            nc.vector.tensor_scalar(negneq, eqP, 1.0, -1.0, Alu.subtract,
                                    Alu.mult)
            z = singles.tile([A, B], f32)
            nc.vector.tensor_scalar(z, pw_ps, -1.0, None, Alu.mult)
            nc.vector.tensor_mul(z, z, negneq)
            nc.vector.tensor_add(z, z, penN)

            zTs = singles.tile([128, NT, A], f32)
            for j in range(NT):
                zt_ps = spsum.tile([128, A], f32)
                nc.tensor.transpose(zt_ps, z[:, j * 128:(j + 1) * 128], ident)
                nc.scalar.copy(zTs[:, j, :], zt_ps)

            Sacc = singles.tile([128, A * NT], f32)
            Cacc = singles.tile([128, A * NT], f32)

            for a in range(A):
                wb_ps = wpsum.tile([128, B], f32)
                nc.tensor.matmul(
                    wb_ps, lhsT=ident[:, a:a + 1].broadcast_to([A, 128]), rhs=w)
                for j in range(NT):
                    col = a * NT + j
                    r = rpool.tile([128, B], bf16)
                    nc.scalar.activation(
                        out=r, in_=wb_ps, func=Act.Relu,
                        bias=zTs[:, j, a:a + 1], scale=1.0,
                        accum_out=Sacc[:, col:col + 1])
                    m = mpool.tile([128, B], bf16)
                    nc.vector.tensor_scalar(
                        m, r, EPS, None, Alu.is_gt, Alu.add,
                        accum_out=Cacc[:, col:col + 1])

            SC = singles.tile([128, 2], f32)
            nc.vector.tensor_reduce(SC[:, 0:1], Sacc, Ax.X, Alu.add)
            nc.vector.tensor_reduce(SC[:, 1:2], Cacc, Ax.X, Alu.add)
            fin_ps = spsum.tile([2, 1], f32)
            nc.tensor.matmul(fin_ps, lhsT=SC, rhs=ones_c)
            fin_sb = singles.tile([2, 1], f32)
            nc.scalar.copy(fin_sb, fin_ps)
            nc.sync.dma_start(
                out=out_d.ap().rearrange("(p o) -> p o", o=1), in_=fin_sb)

    nc.compile()
    return nc


def _get_nc(variant):
    key = f"nc_{variant}"
    if key not in _CACHE:
        _CACHE[key] = _build_win() if variant == "win" else _build_dense()
    return _CACHE[key]


def _selc():
    if "selc" not in _CACHE:
        np_ = A // 2
        s = np.zeros((A, np_, 2, 64), np.float16)
        for ar in range(np_):
            for h in range(2):
                s[ar + 32 * h, ar, h, :] = 1.0
        _CACHE["selc"] = np.ascontiguousarray(s.reshape(A, -1))
    return _CACHE["selc"]


def _prep(labels, image_embeddings, sentence_embeddings):
    """Class-sort the batch; build per-core input maps."""
    labels = np.ascontiguousarray(labels)
    img = np.ascontiguousarray(image_embeddings, dtype=np.float32)
    sent = np.ascontiguousarray(sentence_embeddings, dtype=np.float32)
    counts = np.bincount(labels.astype(np.int64))
    variant = "win" if counts.max() <= MAXC_WIN else "dense"

    perm = np.argsort(labels, kind="stable")
    labs = labels[perm].astype(np.float32)
    imgT = np.ascontiguousarray(img[perm].T)    # [D, B]
    sentT = np.ascontiguousarray(sent[perm].T)  # [D, B]
    if variant == "win":
        imgT = imgT.astype(np.float16)
        sentT = sentT.astype(np.float16)

    maps = []
    for i in range(NCORES):
        c0 = i * A
        m = {"labc": np.ascontiguousarray(labs[c0:c0 + A])}
        if variant == "win":
            # rotate the sentence axis so this core's 128-wide window
            # [c0-32, c0+96) lands at columns [0, W)
            rot = (np.arange(B) + c0 - 32) % B
            m["packT"] = np.ascontiguousarray(
                np.concatenate([imgT[:, c0:c0 + A], sentT[:, rot]], axis=1))
            m["labf"] = np.ascontiguousarray(labs[rot]).astype(np.float16)
        else:
            m["imgT"] = np.ascontiguousarray(imgT[:, c0:c0 + A])
            m["sentT"] = sentT
            m["labf"] = labs
        maps.append(m)
    return variant, maps


def run_all(labels, image_embeddings, sentence_embeddings, trace=False):
    from concourse.bass_utils import run_bass_kernel_spmd
    variant, maps = _prep(labels, image_embeddings, sentence_embeddings)
    nc = _get_nc(variant)
    res = run_bass_kernel_spmd(nc, maps, list(range(NCORES)), trace=trace)
    parts = np.stack([res.results[i]["out"] for i in range(NCORES)])
    s = float(parts[:, 0].sum())
    c = float(parts[:, 1].sum())
    loss = np.float32(s / (c + EPS))
    return np.asarray(loss, dtype=np.float32), res


def kernel(labels, image_embeddings, sentence_embeddings):
    out, _ = run_all(labels, image_embeddings, sentence_embeddings)
    return out



# revision 13
# speedup vs baseline: 1.5971x; 1.5971x over previous
"""Trainium2 Bass kernel for BatchAll triplet loss.

Reference computation (B=512, D=1024):
    pw = img @ sent.T                                  [B, B]
    t[a,p,n] = pw[a,p] - pw[a,n] + margin
    valid[a,p,n] = (lab[a]==lab[p]) & (lab[a]!=lab[n])
    loss = sum(relu(valid*t)) / (count(valid*t > EPS) + EPS)

Strategy ("pair" variant): the batch is class-sorted on the host (a pure
permutation; the loss is permutation invariant) and anchors are sharded
across 8 cores (64 each). The host enumerates the actual valid (a, p)
pairs per core (sum over anchors of their class size, ~320) and bakes
all label-derived structure into small DMA'd constants:

  - packT  [128, 8, 576] fp8e4: imgT core slice ++ rotated sentT,
    k-major for DoubleRow (2x) matmuls.
  - pen    [64, 512] fp16: margin - 30000 * (lab[a]==lab[n]) on the
    rotated n axis (row = local anchor).
  - selM   [128, NP] fp16: column i holds -1 at the pair's local anchor
    row (rows 0:64, picks -pw) and +1 at 64+row (picks pen).
  - m1hot  [128, NPT, 128] fp16: one-hot of the pair's positive column
    inside the core's 128-wide rotated window.

The pairwise GEMM runs in fp8 (input quantization costs ~2e-3 rel), but
pw is evacuated to an fp16 operand: an fp8 stack would put the
quantization grid step (~8 at |pw|~100) above the margin 0.2 and
systematically inflate the count.

Device (per core):
    pw_ps[64, 512]  = img-slice @ rotated-sentT   (4 fp8 DoubleRow MMs)
    stack[0:64, :]  = pw (fp16, ACT copy)
    stack[64:128,:] = pen (DMA)
    per tile j of 128 pairs:
      rb_ps  = selM[0:64] @ pw[:, 0:128]          -> -pw row bcast
      wcol_j = sum(rb_ps * m1hot_j) on DVE        -> pw[a_i, p_i]
      zb_ps  = selM_j.T @ stack                   -> -pw[a_i, n] + pen
      ACT: r = relu(zb + wcol_j), accum_out -> Sacc col j
      DVE: count = (r > EPS), accum_out -> Cacc col j
    final: column-reduce Sacc/Cacc, [2,1] fp32 matmul with ones, DMA.

Host combines the 8 (sum, count) pairs: loss = sum / count.
A dense fallback handles pathological label distributions
(max class size > 33).
"""

import numpy as np
from contextlib import ExitStack

import ml_dtypes

B = 512
D = 1024
NCORES = 8
A = B // NCORES   # 64 anchors per core
KT = D // 128     # 8 contraction tiles
NT = B // 128     # 4 n-tiles per anchor (dense variant)
W = 128           # per-core sentence window width
MARGIN = 0.2
EPS = 1e-16
BIG = 1e30
PEN = -30000.0    # fp16-representable mask penalty
MAXC_PAIR = 33    # pair variant valid iff max class size <= this

FP8 = ml_dtypes.float8_e4m3
BF16 = ml_dtypes.bfloat16

_CACHE = {}


def _build_pair(npt):
    """Class-sorted exact-pair kernel (primary)."""
    import concourse.mybir as mybir
    import concourse.tile as tile
    from concourse import bacc

    f32 = mybir.dt.float32
    bf16 = mybir.dt.bfloat16
    fp16 = mybir.dt.float16
    fp8 = mybir.dt.float8e4
    Alu = mybir.AluOpType
    Act = mybir.ActivationFunctionType
    Ax = mybir.AxisListType
    DR = mybir.MatmulPerfMode.DoubleRow

    NP = npt * 128

    nc = bacc.Bacc("TRN2", target_bir_lowering=False, debug=False,
                   num_devices=NCORES)

    packT_d = nc.dram_tensor("packT", [D, A + B], fp8, kind="ExternalInput")
    pen_d = nc.dram_tensor("pen", [A, B], fp16, kind="ExternalInput")
    selM_d = nc.dram_tensor("selM", [128, NP], fp16, kind="ExternalInput")
    m1hot_d = nc.dram_tensor("m1hot", [128, npt * W], fp16,
                             kind="ExternalInput")
    out_d = nc.dram_tensor("out", [2], f32, kind="ExternalOutput")

    with tile.TileContext(nc) as tc:
        with ExitStack() as ctx:
            singles = ctx.enter_context(tc.tile_pool(name="singles", bufs=1))
            rpool = ctx.enter_context(tc.tile_pool(name="rpool", bufs=3))
            mpool = ctx.enter_context(tc.tile_pool(name="mpool", bufs=3))
            ppsum = ctx.enter_context(
                tc.tile_pool(name="ppsum", bufs=1, space="PSUM"))
            rbpsum = ctx.enter_context(
                tc.tile_pool(name="rbpsum", bufs=2, space="PSUM"))
            zpsum = ctx.enter_context(
                tc.tile_pool(name="zpsum", bufs=3, space="PSUM"))
            fpsum = ctx.enter_context(
                tc.tile_pool(name="fpsum", bufs=1, space="PSUM"))

            # ---- tiles ----
            packT = singles.tile([128, KT, A + B], fp8)
            stack = singles.tile([128, B], fp16)
            selM = singles.tile([128, NP], fp16)
            m1hot = singles.tile([128, npt, W], fp16)
            wcol = singles.tile([128, npt], f32)
            Sacc = singles.tile([128, npt], f32)
            Cacc = singles.tile([128, npt], f32)
            ones_c = singles.tile([128, 1], f32)
            nc.vector.memset(ones_c, 1.0)

            # ---- input DMAs: packT k-tiles split across both HW queues,
            # pair (2k, 2k+1) lands first on each so DoubleRow MM k can
            # start; constants trail on the same queues ----
            packT_v = packT_d.ap().rearrange("(t p) m -> p t m", p=128)
            for kt in range(KT):
                eng = nc.sync if kt % 2 == 0 else nc.scalar
                eng.dma_start(out=packT[:, kt, :], in_=packT_v[:, kt, :])
            nc.sync.dma_start(out=selM, in_=selM_d.ap())
            nc.scalar.dma_start(
                out=m1hot,
                in_=m1hot_d.ap().rearrange("p (t w) -> p t w", t=npt))
            nc.sync.dma_start(out=stack[A:128, :], in_=pen_d.ap())

            # ---- pairwise rows: 4 DoubleRow fp8 matmuls ----
            pw_ps = ppsum.tile([A, B], f32)
            for k in range(KT // 2):
                nc.tensor.matmul(pw_ps,
                                 lhsT=packT[:, 2 * k:2 * k + 2, 0:A],
                                 rhs=packT[:, 2 * k:2 * k + 2, A:A + B],
                                 start=(k == 0), stop=(k == KT // 2 - 1),
                                 perf_mode=DR)

            # ---- evacuate pw into the fp16 matmul operand ----
            nc.scalar.activation(out=stack[0:A, :], in_=pw_ps, func=Act.Copy,
                                 bias=0.0, scale=1.0)

            # ---- main loop: one tile of 128 pairs per iteration ----
            for j in range(npt):
                js = slice(128 * j, 128 * (j + 1))
                rb_ps = rbpsum.tile([128, W], f32)
                nc.tensor.matmul(rb_ps, lhsT=selM[0:A, js],
                                 rhs=stack[0:A, 0:W])
                wtmp = mpool.tile([128, W], bf16)
                nc.vector.scalar_tensor_tensor(
                    wtmp, rb_ps, 1.0, m1hot[:, j, :], Alu.mult, Alu.mult,
                    accum_out=wcol[:, j:j + 1])
                zb_ps = zpsum.tile([128, B], f32)
                nc.tensor.matmul(zb_ps, lhsT=selM[:, js], rhs=stack)
                r = rpool.tile([128, B], bf16)
                nc.scalar.activation(
                    out=r, in_=zb_ps, func=Act.Relu,
                    bias=wcol[:, j:j + 1], scale=1.0,
                    accum_out=Sacc[:, j:j + 1])
                m = mpool.tile([128, B], bf16)
                nc.vector.tensor_scalar(
                    m, r, EPS, None, Alu.is_gt, Alu.add,
                    accum_out=Cacc[:, j:j + 1])

            # ---- final reduce ----
            SC = singles.tile([128, 2], f32)
            nc.vector.tensor_reduce(SC[:, 0:1], Sacc, Ax.X, Alu.add)
            nc.vector.tensor_reduce(SC[:, 1:2], Cacc, Ax.X, Alu.add)
            fin_ps = fpsum.tile([2, 1], f32)
            nc.tensor.matmul(fin_ps, lhsT=SC, rhs=ones_c)
            fin_sb = singles.tile([2, 1], f32)
            nc.scalar.copy(fin_sb, fin_ps)
            nc.sync.dma_start(
                out=out_d.ap().rearrange("(p o) -> p o", o=1), in_=fin_sb)

    nc.compile()
    return nc


def _build_dense():
    """Dense fallback (no class-size assumption)."""
    import concourse.mybir as mybir
    import concourse.tile as tile
    from concourse import bacc
    from concourse.masks import make_identity

    f32 = mybir.dt.float32
    bf16 = mybir.dt.bfloat16
    Alu = mybir.AluOpType
    Act = mybir.ActivationFunctionType
    Ax = mybir.AxisListType

    nc = bacc.Bacc("TRN2", target_bir_lowering=False, debug=False,
                   num_devices=NCORES)

    imgT_d = nc.dram_tensor("imgT", [D, A], f32, kind="ExternalInput")
    sentT_d = nc.dram_tensor("sentT", [D, B], f32, kind="ExternalInput")
    labf_d = nc.dram_tensor("labf", [B], bf16, kind="ExternalInput")
    labc_d = nc.dram_tensor("labc", [A], f32, kind="ExternalInput")
    out_d = nc.dram_tensor("out", [2], f32, kind="ExternalOutput")

    with tile.TileContext(nc) as tc:
        with ExitStack() as ctx:
            singles = ctx.enter_context(tc.tile_pool(name="singles", bufs=1))
            rpool = ctx.enter_context(tc.tile_pool(name="rpool", bufs=6))
            mpool = ctx.enter_context(tc.tile_pool(name="mpool", bufs=6))
            spsum = ctx.enter_context(
                tc.tile_pool(name="spsum", bufs=1, space="PSUM"))
            wpsum = ctx.enter_context(
                tc.tile_pool(name="wpsum", bufs=3, space="PSUM"))

            ones_r = singles.tile([1, 128], f32)
            nc.vector.memset(ones_r, 1.0)
            ones_c = singles.tile([128, 1], f32)
            nc.vector.memset(ones_c, 1.0)
            ident = singles.tile([64, 64], f32)
            make_identity(nc, ident)

            imgT = singles.tile([128, KT, A], f32)
            nc.sync.dma_start(
                out=imgT, in_=imgT_d.ap().rearrange("(t p) m -> p t m", p=128))
            sentT = singles.tile([128, KT, B], f32)
            nc.sync.dma_start(
                out=sentT, in_=sentT_d.ap().rearrange("(t p) m -> p t m", p=128))
            lab_row = singles.tile([1, B], f32)
            nc.sync.dma_start(
                out=lab_row, in_=labf_d.ap().rearrange("(o b) -> o b", o=1))
            labc_col = singles.tile([A, 1], f32)
            nc.sync.dma_start(
                out=labc_col, in_=labc_d.ap().rearrange("(a o) -> a o", o=1))

            pw_ps = spsum.tile([A, B], f32)
            for kt in range(KT):
                nc.tensor.matmul(pw_ps, lhsT=imgT[:, kt, :], rhs=sentT[:, kt, :],
                                 start=(kt == 0), stop=(kt == KT - 1))

            labB_ps = spsum.tile([A, B], f32)
            nc.tensor.matmul(labB_ps, lhsT=ones_r[:, :A], rhs=lab_row)
            eqP = singles.tile([A, B], f32)
            nc.vector.tensor_scalar(eqP, labB_ps, labc_col, None, Alu.is_equal)
            penP = singles.tile([A, B], f32)
            nc.vector.tensor_scalar(penP, eqP, 1.0, BIG, Alu.subtract, Alu.mult)
            penN = singles.tile([A, B], f32)
            nc.vector.tensor_scalar(penN, eqP, -BIG, None, Alu.mult)

            w = singles.tile([A, B], f32)
            nc.vector.tensor_scalar(w, pw_ps, MARGIN, None, Alu.add)
            nc.vector.tensor_mul(w, w, eqP)
            nc.vector.tensor_add(w, w, penP)
            negneq = singles.tile([A, B], f32)
            nc.vector.tensor_scalar(negneq, eqP, 1.0, -1.0, Alu.subtract,
                                    Alu.mult)
            z = singles.tile([A, B], f32)
            nc.vector.tensor_scalar(z, pw_ps, -1.0, None, Alu.mult)
            nc.vector.tensor_mul(z, z, negneq)
            nc.vector.tensor_add(z, z, penN)

            zTs = singles.tile([128, NT, A], f32)
            for j in range(NT):
                zt_ps = spsum.tile([128, A], f32)
                nc.tensor.transpose(zt_ps, z[:, j * 128:(j + 1) * 128], ident)
                nc.scalar.copy(zTs[:, j, :], zt_ps)

            Sacc = singles.tile([128, A * NT], f32)
            Cacc = singles.tile([128, A * NT], f32)

            for a in range(A):
                wb_ps = wpsum.tile([128, B], f32)
                nc.tensor.matmul(
                    wb_ps, lhsT=ident[:, a:a + 1].broadcast_to([A, 128]), rhs=w)
                for j in range(NT):
                    col = a * NT + j
                    r = rpool.tile([128, B], bf16)
                    nc.scalar.activation(
                        out=r, in_=wb_ps, func=Act.Relu,
                        bias=zTs[:, j, a:a + 1], scale=1.0,
                        accum_out=Sacc[:, col:col + 1])
                    m = mpool.tile([128, B], bf16)
                    nc.vector.tensor_scalar(
                        m, r, EPS, None, Alu.is_gt, Alu.add,
                        accum_out=Cacc[:, col:col + 1])

            SC = singles.tile([128, 2], f32)
            nc.vector.tensor_reduce(SC[:, 0:1], Sacc, Ax.X, Alu.add)
            nc.vector.tensor_reduce(SC[:, 1:2], Cacc, Ax.X, Alu.add)
            fin_ps = spsum.tile([2, 1], f32)
            nc.tensor.matmul(fin_ps, lhsT=SC, rhs=ones_c)
            fin_sb = singles.tile([2, 1], f32)
            nc.scalar.copy(fin_sb, fin_ps)
            nc.sync.dma_start(
                out=out_d.ap().rearrange("(p o) -> p o", o=1), in_=fin_sb)

    nc.compile()
    return nc


def _get_nc(variant, npt=0):
    key = f"nc_{variant}_{npt}"
    if key not in _CACHE:
        _CACHE[key] = (_build_pair(npt) if variant == "pair"
                       else _build_dense())
    return _CACHE[key]


def _prep(labels, image_embeddings, sentence_embeddings):
    """Class-sort the batch; build per-core input maps."""
    labels = np.ascontiguousarray(labels)
    img = np.ascontiguousarray(image_embeddings, dtype=np.float32)
    sent = np.ascontiguousarray(sentence_embeddings, dtype=np.float32)
    counts = np.bincount(labels.astype(np.int64))
    variant = "pair" if counts.max() <= MAXC_PAIR else "dense"

    perm = np.argsort(labels, kind="stable")
    labs = labels[perm]

    if variant == "dense":
        labsf = labs.astype(np.float32)
        imgT = np.ascontiguousarray(img[perm].T)    # [D, B]
        sentT = np.ascontiguousarray(sent[perm].T)  # [D, B]
        maps = []
        for i in range(NCORES):
            c0 = i * A
            maps.append({
                "labc": np.ascontiguousarray(labsf[c0:c0 + A]),
                "imgT": np.ascontiguousarray(imgT[:, c0:c0 + A]),
                "sentT": sentT,
                "labf": labsf.astype(BF16),
            })
        return variant, 0, maps

    img8T = np.ascontiguousarray(img[perm].astype(FP8).T)    # [D, B]
    sent8T = np.ascontiguousarray(sent[perm].astype(FP8).T)  # [D, B]

    # class run start/size for every sorted position
    run_start = np.zeros(B, np.int64)
    run_size = np.zeros(B, np.int64)
    pos = 0
    for lab, sz in zip(*np.unique(labs, return_counts=True)):
        idx = slice(pos, pos + sz)
        run_start[idx] = pos
        run_size[idx] = sz
        pos += sz

    # per-core pair lists (local anchor, local window column)
    pair_la, pair_lp = [], []
    for c in range(NCORES):
        c0 = c * A
        las, lps = [], []
        for la in range(A):
            a = c0 + la
            st, sz = run_start[a], run_size[a]
            for p in range(st, st + sz):
                las.append(la)
                lps.append(p - c0 + 32)  # in [0, 128) since sz <= 33
        pair_la.append(np.array(las))
        pair_lp.append(np.array(lps))
    npt = max(-(-len(x) // 128) for x in pair_la)
    NP = npt * 128

    maps = []
    for c in range(NCORES):
        c0 = c * A
        rot = (np.arange(B) + c0 - 32) % B
        packT = np.concatenate([img8T[:, c0:c0 + A], sent8T[:, rot]], axis=1)
        eq = (labs[c0:c0 + A, None] == labs[rot][None, :])
        pen = np.where(eq, np.float32(PEN), np.float32(MARGIN))
        las, lps = pair_la[c], pair_lp[c]
        n = len(las)
        selM = np.zeros((128, NP), np.float32)
        selM[las, np.arange(n)] = -1.0
        selM[A + las, np.arange(n)] = 1.0
        # rb uses the -1 selector half (base partition 0), so the one-hot
        # carries -1 to restore +pw in the gather
        m1hot = np.zeros((128, npt, W), np.float32)
        m1hot[np.arange(n) % 128, np.arange(n) // 128, lps] = -1.0
        maps.append({
            "packT": np.ascontiguousarray(packT),
            "pen": np.ascontiguousarray(pen.astype(np.float16)),
            "selM": np.ascontiguousarray(selM.astype(np.float16)),
            "m1hot": np.ascontiguousarray(
                m1hot.reshape(128, npt * W).astype(np.float16)),
        })
    return variant, npt, maps


def run_all(labels, image_embeddings, sentence_embeddings, trace=False):
    from concourse.bass_utils import run_bass_kernel_spmd
    variant, npt, maps = _prep(labels, image_embeddings, sentence_embeddings)
    nc = _get_nc(variant, npt)
    res = run_bass_kernel_spmd(nc, maps, list(range(NCORES)), trace=trace)
    parts = np.stack([res.results[i]["out"] for i in range(NCORES)])
    s = float(parts[:, 0].sum())
    c = float(parts[:, 1].sum())
    loss = np.float32(s / (c + EPS))
    return np.asarray(loss, dtype=np.float32), res


def kernel(labels, image_embeddings, sentence_embeddings):
    out, _ = run_all(labels, image_embeddings, sentence_embeddings)
    return out


# revision 20
# speedup vs baseline: 1.8430x; 1.1539x over previous
"""Trainium2 Bass kernel for BatchAll triplet loss.

Reference computation (B=512, D=1024):
    pw = img @ sent.T                                  [B, B]
    t[a,p,n] = pw[a,p] - pw[a,n] + margin
    valid[a,p,n] = (lab[a]==lab[p]) & (lab[a]!=lab[n])
    loss = sum(relu(valid*t)) / (count(valid*t > EPS) + EPS)

Strategy ("pair" variant): the batch is class-sorted on the host (a pure
permutation; the loss is permutation invariant) and anchors are sharded
across 8 cores (64 each). The host enumerates the actual valid (a, p)
pairs per core (sum over anchors of their class size, ~320) and bakes
all label-derived structure into small DMA'd constants:

  - packT  [128, 8, 576] fp8e4: imgT core slice ++ rotated sentT,
    k-major for DoubleRow (2x) matmuls.
  - pen    [64, 512] fp16: margin - 30000 * (lab[a]==lab[n]) on the
    rotated n axis (row = local anchor).
  - selM   [128, NP] fp16: column i holds -1 at the pair's local anchor
    row (rows 0:64, picks -pw) and +1 at 64+row (picks pen).
  - m1hot  [128, NPT, 128] fp16: one-hot of the pair's positive column
    inside the core's 128-wide rotated window.

The pairwise GEMM runs in fp8 (input quantization costs ~2e-3 rel), but
pw is evacuated to an fp16 operand: an fp8 stack would put the
quantization grid step (~8 at |pw|~100) above the margin 0.2 and
systematically inflate the count.

Device (per core):
    pw_ps[64, 512]  = img-slice @ rotated-sentT   (4 fp8 DoubleRow MMs)
    stack[0:64, :]  = pw (fp16, ACT copy)
    stack[64:128,:] = pen (DMA)
    per tile j of 128 pairs:
      rb_ps  = selM[0:64] @ pw[:, 0:128]          -> -pw row bcast
      wcol_j = sum(rb_ps * m1hot_j) on DVE        -> pw[a_i, p_i]
      zb_ps  = selM_j.T @ stack                   -> -pw[a_i, n] + pen
      ACT: r = relu(zb + wcol_j), accum_out -> Sacc col j
      DVE: count = (r > EPS), accum_out -> Cacc col j
    final: column-reduce Sacc/Cacc, [2,1] fp32 matmul with ones, DMA.

Host combines the 8 (sum, count) pairs: loss = sum / count.
A dense fallback handles pathological label distributions
(max class size > 33).
"""

import numpy as np
from contextlib import ExitStack

import ml_dtypes

B = 512
D = 1024
NCORES = 8
A = B // NCORES   # 64 anchors per core
KT = D // 128     # 8 contraction tiles
NT = B // 128     # 4 n-tiles per anchor (dense variant)
W = 128           # per-core sentence window width
MARGIN = 0.2
EPS = 1e-16
BIG = 1e30
PEN = -30000.0    # fp16-representable mask penalty
MAXC_PAIR = 33    # pair variant valid iff max class size <= this

FP8 = ml_dtypes.float8_e4m3
BF16 = ml_dtypes.bfloat16

_CACHE = {}


def _build_pair(npt):
    """Class-sorted exact-pair kernel (primary)."""
    import concourse.mybir as mybir
    import concourse.tile as tile
    from concourse import bacc

    f32 = mybir.dt.float32
    bf16 = mybir.dt.bfloat16
    fp16 = mybir.dt.float16
    fp8 = mybir.dt.float8e4
    Alu = mybir.AluOpType
    Act = mybir.ActivationFunctionType
    Ax = mybir.AxisListType
    DR = mybir.MatmulPerfMode.DoubleRow

    NP = npt * 128

    nc = bacc.Bacc("TRN2", target_bir_lowering=False, debug=False,
                   num_devices=NCORES)

    # packT partition-major: row p holds all KT k-tiles contiguously so a
    # DMA moves large per-partition elements (descriptor-overhead-bound
    # otherwise); smc packs selM + m1hot into one transfer.
    packT_d = nc.dram_tensor("packT", [128, KT * (A + B)], fp8,
                             kind="ExternalInput")
    pen_d = nc.dram_tensor("pen", [A, B], fp16, kind="ExternalInput")
    smc_d = nc.dram_tensor("smc", [128, 2 * NP], fp16, kind="ExternalInput")
    out_d = nc.dram_tensor("out", [2], f32, kind="ExternalOutput")

    with tile.TileContext(nc) as tc:
        with ExitStack() as ctx:
            singles = ctx.enter_context(tc.tile_pool(name="singles", bufs=1))
            rpool = ctx.enter_context(tc.tile_pool(name="rpool", bufs=1))
            mpool = ctx.enter_context(tc.tile_pool(name="mpool", bufs=1))
            ppsum = ctx.enter_context(
                tc.tile_pool(name="ppsum", bufs=1, space="PSUM"))
            zpsum = ctx.enter_context(
                tc.tile_pool(name="zpsum", bufs=1, space="PSUM"))
            fpsum = ctx.enter_context(
                tc.tile_pool(name="fpsum", bufs=1, space="PSUM"))

            # ---- tiles ----
            packT = singles.tile([128, KT, A + B], fp8)
            stack = singles.tile([128, B], fp16)
            combo = singles.tile([128, 2, npt, W], fp16)  # selM / m1hot
            wcol = singles.tile([128, npt], f32)
            acc = singles.tile([128, 2, npt], f32)        # sums / counts
            ones_c = singles.tile([128, 1], f32)
            nc.vector.memset(ones_c, 1.0)

            # ---- input DMAs: k-tile pairs alternate between the two HW
            # queues so DoubleRow MM k can start as its pair lands ----
            packT_v = packT_d.ap().rearrange("p (t m) -> p t m", t=KT)
            nc.sync.dma_start(out=packT[:, 0:2, :], in_=packT_v[:, 0:2, :])
            nc.scalar.dma_start(out=packT[:, 2:4, :], in_=packT_v[:, 2:4, :])
            nc.sync.dma_start(out=packT[:, 4:6, :], in_=packT_v[:, 4:6, :])
            nc.scalar.dma_start(out=packT[:, 6:8, :], in_=packT_v[:, 6:8, :])
            nc.sync.dma_start(
                out=combo,
                in_=smc_d.ap().rearrange("p (h t w) -> p h t w", h=2, t=npt))
            nc.scalar.dma_start(out=stack[A:128, :], in_=pen_d.ap())

            # ---- pairwise rows: 4 DoubleRow fp8 matmuls ----
            pw_ps = ppsum.tile([A, B], f32)
            for k in range(KT // 2):
                nc.tensor.matmul(pw_ps,
                                 lhsT=packT[:, 2 * k:2 * k + 2, 0:A],
                                 rhs=packT[:, 2 * k:2 * k + 2, A:A + B],
                                 start=(k == 0), stop=(k == KT // 2 - 1),
                                 perf_mode=DR)

            # ---- evacuate pw into the fp16 matmul operand ----
            nc.scalar.activation(out=stack[0:A, :], in_=pw_ps, func=Act.Copy,
                                 bias=0.0, scale=1.0)

            # ---- selector matmuls: zb = -pw[a_i] + pen[a_i] ----
            zb = []
            for j in range(npt):
                zb_ps = zpsum.tile([128, B], f32, name=f"zb{j}")
                nc.tensor.matmul(zb_ps, lhsT=combo[:, 0, j, :], rhs=stack)
                zb.append(zb_ps)

            # ---- per tile: wcol gather (DVE), relu+sum (ACT/DVE),
            # count (Pool/DVE). wcol = PEN - zb[i, lp_i] = pw[a_i, p_i]
            # via (zb - PEN) * (-one-hot) with free-axis accumulate ----
            r = [rpool.tile([128, B], bf16, name=f"r{j}") for j in range(npt)]
            for j in range(npt):
                wtmp = mpool.tile([128, W], bf16, name=f"wtmp{j}")
                nc.vector.scalar_tensor_tensor(
                    wtmp, zb[j][:, 0:W], PEN, combo[:, 1, j, :],
                    Alu.subtract, Alu.mult, accum_out=wcol[:, j:j + 1])
            for j in range(npt):
                nc.scalar.activation(
                    out=r[j], in_=zb[j], func=Act.Relu, bias=wcol[:, j:j + 1],
                    scale=1.0, accum_out=acc[:, 0, j:j + 1])
            for j in range(npt):
                m = mpool.tile([128, B], bf16, name=f"m{j}")
                nc.vector.tensor_scalar(
                    m, r[j], EPS, None, Alu.is_gt, Alu.add,
                    accum_out=acc[:, 1, j:j + 1])

            # ---- final reduce ----
            SC = singles.tile([128, 2], f32)
            nc.vector.tensor_reduce(SC, acc, Ax.X, Alu.add)
            fin_ps = fpsum.tile([2, 1], f32)
            nc.tensor.matmul(fin_ps, lhsT=SC, rhs=ones_c)
            fin_sb = singles.tile([2, 1], f32)
            nc.scalar.copy(fin_sb, fin_ps)
            nc.sync.dma_start(
                out=out_d.ap().rearrange("(p o) -> p o", o=1), in_=fin_sb)

    nc.compile()
    return nc


def _build_dense():
    """Dense fallback (no class-size assumption)."""
    import concourse.mybir as mybir
    import concourse.tile as tile
    from concourse import bacc
    from concourse.masks import make_identity

    f32 = mybir.dt.float32
    bf16 = mybir.dt.bfloat16
    Alu = mybir.AluOpType
    Act = mybir.ActivationFunctionType
    Ax = mybir.AxisListType

    nc = bacc.Bacc("TRN2", target_bir_lowering=False, debug=False,
                   num_devices=NCORES)

    imgT_d = nc.dram_tensor("imgT", [D, A], f32, kind="ExternalInput")
    sentT_d = nc.dram_tensor("sentT", [D, B], f32, kind="ExternalInput")
    labf_d = nc.dram_tensor("labf", [B], bf16, kind="ExternalInput")
    labc_d = nc.dram_tensor("labc", [A], f32, kind="ExternalInput")
    out_d = nc.dram_tensor("out", [2], f32, kind="ExternalOutput")

    with tile.TileContext(nc) as tc:
        with ExitStack() as ctx:
            singles = ctx.enter_context(tc.tile_pool(name="singles", bufs=1))
            rpool = ctx.enter_context(tc.tile_pool(name="rpool", bufs=6))
            mpool = ctx.enter_context(tc.tile_pool(name="mpool", bufs=6))
            spsum = ctx.enter_context(
                tc.tile_pool(name="spsum", bufs=1, space="PSUM"))
            wpsum = ctx.enter_context(
                tc.tile_pool(name="wpsum", bufs=3, space="PSUM"))

            ones_r = singles.tile([1, 128], f32)
            nc.vector.memset(ones_r, 1.0)
            ones_c = singles.tile([128, 1], f32)
            nc.vector.memset(ones_c, 1.0)
            ident = singles.tile([64, 64], f32)
            make_identity(nc, ident)

            imgT = singles.tile([128, KT, A], f32)
            nc.sync.dma_start(
                out=imgT, in_=imgT_d.ap().rearrange("(t p) m -> p t m", p=128))
            sentT = singles.tile([128, KT, B], f32)
            nc.sync.dma_start(
                out=sentT, in_=sentT_d.ap().rearrange("(t p) m -> p t m", p=128))
            lab_row = singles.tile([1, B], f32)
            nc.sync.dma_start(
                out=lab_row, in_=labf_d.ap().rearrange("(o b) -> o b", o=1))
            labc_col = singles.tile([A, 1], f32)
            nc.sync.dma_start(
                out=labc_col, in_=labc_d.ap().rearrange("(a o) -> a o", o=1))

            pw_ps = spsum.tile([A, B], f32)
            for kt in range(KT):
                nc.tensor.matmul(pw_ps, lhsT=imgT[:, kt, :], rhs=sentT[:, kt, :],
                                 start=(kt == 0), stop=(kt == KT - 1))

            labB_ps = spsum.tile([A, B], f32)
            nc.tensor.matmul(labB_ps, lhsT=ones_r[:, :A], rhs=lab_row)
            eqP = singles.tile([A, B], f32)
            nc.vector.tensor_scalar(eqP, labB_ps, labc_col, None, Alu.is_equal)
            penP = singles.tile([A, B], f32)
            nc.vector.tensor_scalar(penP, eqP, 1.0, BIG, Alu.subtract, Alu.mult)
            penN = singles.tile([A, B], f32)
            nc.vector.tensor_scalar(penN, eqP, -BIG, None, Alu.mult)

            w = singles.tile([A, B], f32)
            nc.vector.tensor_scalar(w, pw_ps, MARGIN, None, Alu.add)
            nc.vector.tensor_mul(w, w, eqP)
            nc.vector.tensor_add(w, w, penP)
            negneq = singles.tile([A, B], f32)
            nc.vector.tensor_scalar(negneq, eqP, 1.0, -1.0, Alu.subtract,
                                    Alu.mult)
            z = singles.tile([A, B], f32)
            nc.vector.tensor_scalar(z, pw_ps, -1.0, None, Alu.mult)
            nc.vector.tensor_mul(z, z, negneq)
            nc.vector.tensor_add(z, z, penN)

            zTs = singles.tile([128, NT, A], f32)
            for j in range(NT):
                zt_ps = spsum.tile([128, A], f32)
                nc.tensor.transpose(zt_ps, z[:, j * 128:(j + 1) * 128], ident)
                nc.scalar.copy(zTs[:, j, :], zt_ps)

            Sacc = singles.tile([128, A * NT], f32)
            Cacc = singles.tile([128, A * NT], f32)

            for a in range(A):
                wb_ps = wpsum.tile([128, B], f32)
                nc.tensor.matmul(
                    wb_ps, lhsT=ident[:, a:a + 1].broadcast_to([A, 128]), rhs=w)
                for j in range(NT):
                    col = a * NT + j
                    r = rpool.tile([128, B], bf16)
                    nc.scalar.activation(
                        out=r, in_=wb_ps, func=Act.Relu,
                        bias=zTs[:, j, a:a + 1], scale=1.0,
                        accum_out=Sacc[:, col:col + 1])
                    m = mpool.tile([128, B], bf16)
                    nc.vector.tensor_scalar(
                        m, r, EPS, None, Alu.is_gt, Alu.add,
                        accum_out=Cacc[:, col:col + 1])

            SC = singles.tile([128, 2], f32)
            nc.vector.tensor_reduce(SC[:, 0:1], Sacc, Ax.X, Alu.add)
            nc.vector.tensor_reduce(SC[:, 1:2], Cacc, Ax.X, Alu.add)
            fin_ps = spsum.tile([2, 1], f32)
            nc.tensor.matmul(fin_ps, lhsT=SC, rhs=ones_c)
            fin_sb = singles.tile([2, 1], f32)
            nc.scalar.copy(fin_sb, fin_ps)
            nc.sync.dma_start(
                out=out_d.ap().rearrange("(p o) -> p o", o=1), in_=fin_sb)

    nc.compile()
    return nc


def _get_nc(variant, npt=0):
    key = f"nc_{variant}_{npt}"
    if key not in _CACHE:
        _CACHE[key] = (_build_pair(npt) if variant == "pair"
                       else _build_dense())
    return _CACHE[key]


def _prep(labels, image_embeddings, sentence_embeddings):
    """Class-sort the batch; build per-core input maps."""
    labels = np.ascontiguousarray(labels)
    img = np.ascontiguousarray(image_embeddings, dtype=np.float32)
    sent = np.ascontiguousarray(sentence_embeddings, dtype=np.float32)
    counts = np.bincount(labels.astype(np.int64))
    variant = "pair" if counts.max() <= MAXC_PAIR else "dense"

    perm = np.argsort(labels, kind="stable")
    labs = labels[perm]

    if variant == "dense":
        labsf = labs.astype(np.float32)
        imgT = np.ascontiguousarray(img[perm].T)    # [D, B]
        sentT = np.ascontiguousarray(sent[perm].T)  # [D, B]
        maps = []
        for i in range(NCORES):
            c0 = i * A
            maps.append({
                "labc": np.ascontiguousarray(labsf[c0:c0 + A]),
                "imgT": np.ascontiguousarray(imgT[:, c0:c0 + A]),
                "sentT": sentT,
                "labf": labsf.astype(BF16),
            })
        return variant, 0, maps

    img8T = np.ascontiguousarray(img[perm].astype(FP8).T)    # [D, B]
    sent8T = np.ascontiguousarray(sent[perm].astype(FP8).T)  # [D, B]

    # class run start/size for every sorted position
    run_start = np.zeros(B, np.int64)
    run_size = np.zeros(B, np.int64)
    pos = 0
    for lab, sz in zip(*np.unique(labs, return_counts=True)):
        idx = slice(pos, pos + sz)
        run_start[idx] = pos
        run_size[idx] = sz
        pos += sz

    # per-core pair lists (local anchor, local window column)
    pair_la, pair_lp = [], []
    for c in range(NCORES):
        c0 = c * A
        las, lps = [], []
        for la in range(A):
            a = c0 + la
            st, sz = run_start[a], run_size[a]
            for p in range(st, st + sz):
                las.append(la)
                lps.append(p - c0 + 32)  # in [0, 128) since sz <= 33
        pair_la.append(np.array(las))
        pair_lp.append(np.array(lps))
    npt = max(-(-len(x) // 128) for x in pair_la)
    NP = npt * 128

    maps = []
    for c in range(NCORES):
        c0 = c * A
        rot = (np.arange(B) + c0 - 32) % B
        full = np.concatenate([img8T[:, c0:c0 + A], sent8T[:, rot]], axis=1)
        # partition-major: row p = all KT k-tiles of partition p
        packT = full.reshape(KT, 128, A + B).transpose(1, 0, 2).reshape(
            128, KT * (A + B))
        eq = (labs[c0:c0 + A, None] == labs[rot][None, :])
        pen = np.where(eq, np.float32(PEN), np.float32(MARGIN))
        las, lps = pair_la[c], pair_lp[c]
        n = len(las)
        smc = np.zeros((128, 2, npt, W), np.float32)
        # selM column blocks: -1 at pw row, +1 at pen row of the pair anchor
        smc[las, 0, np.arange(n) // 128, np.arange(n) % 128] = -1.0
        smc[A + las, 0, np.arange(n) // 128, np.arange(n) % 128] = 1.0
        # m1hot: -1 at the pair's window column (wcol = PEN - zb[i, lp])
        smc[np.arange(n) % 128, 1, np.arange(n) // 128, lps] = -1.0
        maps.append({
            "packT": np.ascontiguousarray(packT),
            "pen": np.ascontiguousarray(pen.astype(np.float16)),
            "smc": np.ascontiguousarray(
                smc.reshape(128, 2 * NP).astype(np.float16)),
        })
    return variant, npt, maps


def run_all(labels, image_embeddings, sentence_embeddings, trace=False):
    from concourse.bass_utils import run_bass_kernel_spmd
    variant, npt, maps = _prep(labels, image_embeddings, sentence_embeddings)
    nc = _get_nc(variant, npt)
    res = run_bass_kernel_spmd(nc, maps, list(range(NCORES)), trace=trace)
    parts = np.stack([res.results[i]["out"] for i in range(NCORES)])
    s = float(parts[:, 0].sum())
    c = float(parts[:, 1].sum())
    loss = np.float32(s / (c + EPS))
    return np.asarray(loss, dtype=np.float32), res


def kernel(labels, image_embeddings, sentence_embeddings):
    out, _ = run_all(labels, image_embeddings, sentence_embeddings)
    return out
